# revision 1
# baseline (speedup 1.0000x reference)
"""Trainium2 Bass kernel for nn_Conv_spe_12489764897428.

Math: out[m, c] = sum_hw hs[0, c, h, w] * ms[m, 0, h, w]
  == matmul ms_flat[8, HW] @ hs_flat[191, HW].T with HW = 512*512 = 262144.

Sharding: HW (contraction) axis split across 8 cores; each core computes the
full [8, 191] partial over its 32768-wide HW slice; host sums the partials.

Per-core device kernel (hw slice S = 32768):
  - hs arrives channel-major [191, S]; the PE contracts over partitions, so
    each 128-wide hw block of hs is transposed on-chip ([ch,128hw]->[128hw,ch])
    with PE transpose-mode via an identity, staged through one PSUM bank
    (both channel groups land in disjoint columns of the same bank; start=True
    only clears has_written bits, values persist), then one DVE/ACT copy moves
    it to a zero-padded SBUF tile [128, 256].
  - ms is pre-transposed on the host (it is 4% of the data) into
    [128, S/128, 8] so each 128-hw block gives the stationary lhsT [128, 8].
  - matmul(psum[8, N], lhsT=msT[128, 8], rhs=hsT[128, N]) accumulates over the
    32 blocks of each DMA chunk in PSUM; chunk partials are summed into an
    SBUF accumulator; final [8, 191] DMA'd out.
  - mode "f32r": operands bitcast to float32r; moving dim padded to N=256 where
    the PE streams 1 row/cycle (vs 4 for plain fp32), transposes 1.5 cyc/row.
    mode "f32": plain fp32 everywhere (exact; PE ~2x slower than the DMA
    roofline).
"""

import numpy as np

import concourse.bass as bass
import concourse.mybir as mybir
import concourse.tile as tile
from concourse.masks import make_identity
from concourse.bass_utils import run_bass_kernel_spmd
from concourse.vector_clock import ScopedClock

N_CORES = 8
CH = 191                 # hs channels (band_hs)
MB = 8                   # ms bands (band_ms)
HW = 512 * 512
HW_C = HW // N_CORES     # 32768 hw positions per core
F32 = mybir.dt.float32
F32R = mybir.dt.float32r

# ---------------------------------------------------------------------------
# Workarounds: walrus in this environment encodes at most ONE sync-wait per
# instruction (CTRL and S3_LW struct lowerings reject more with "Too many
# sync wait commands"). Tile freely attaches several. Split them: keep one
# wait on the instruction, hoist the rest onto same-engine NOPs placed just
# before it in the scheduled order.
# ---------------------------------------------------------------------------

_orig_lower_ordered_insts = tile.TileContext._lower_ordered_insts


def _split_multi_waits(nc, blocks):
    for bb, insts in list(blocks.items()):
        new_list = []
        changed = False
        for inst in insts:
            si = getattr(inst, "sync_info", None)
            waits = list(si.on_wait) if si is not None and si.on_wait else []
            if len(waits) > 1:
                si.on_wait = [waits[0]]
                for w in waits[1:]:
                    nop = mybir.InstNoOp(
                        name=nc.get_next_instruction_name(),
                        engine=inst.engine,
                        ins=[],
                        outs=[],
                        sync_info=mybir.SyncInfo(on_wait=[w], on_update=[]),
                        bass_nofuse=True,
                    )
                    nc.register_instruction(nop)
                    new_list.append(nop)
                changed = True
            new_list.append(inst)
        if changed:
            blocks[bb] = new_list


def _patched_lower_ordered_insts(self, postordered_blocks):
    _split_multi_waits(self.nc, postordered_blocks)
    return _orig_lower_ordered_insts(self, postordered_blocks)


tile.TileContext._lower_ordered_insts = _patched_lower_ordered_insts


def _patched_drain_and_barrier(self, tick_clock, wait_clock):
    nop_inst = self.nc.sync.nop(nofuse=True, hint="tail_drain_waits")
    wait_clock.add_sem_waits(
        nop_inst.ins, ScopedClock({None: tick_clock.global_clock})
    )
    si = nop_inst.ins.sync_info
    waits = list(si.on_wait) if si is not None and si.on_wait else []
    if len(waits) > 1:
        si.on_wait = [waits[0]]
        for w in waits[1:]:
            extra = self.nc.sync.nop(nofuse=True, hint="tail_drain_waits")
            esi = extra.ins.sync_info
            if esi is None:
                extra.ins.sync_info = mybir.SyncInfo(on_wait=[w], on_update=[])
            else:
                esi.on_wait = [w]

    self.nc.sync.drain()

    self.nc.all_engine_barrier()
    assert self.sems is not None
    popped = self.nc._tile_sem_poison_stack.pop()
    assert popped is self._sem_poison
    self.nc.clear_and_free_semaphores(list(self.sems.allocated().values()))
    self.nc.all_engine_barrier()


tile.TileContext._drain_and_barrier = _patched_drain_and_barrier


# ---------------------------------------------------------------------------
# Device kernel
# ---------------------------------------------------------------------------


def _emit_body(nc, pools, hs_d, mst_sb, out_d, hw_c, w_chunk, mode, ident):
    (hs0_pool, hs1_pool, hsT_tiles, pt_pool, pacc_pool, acc_pool) = pools
    n_outer = hw_c // w_chunk
    nj = w_chunk // 128
    n_pad = 256 if mode == "f32r" else CH

    acc_sb = acc_pool.tile([MB, n_pad], F32, tag="acc")
    for i in range(n_outer):
        h0 = hs0_pool.tile([128, w_chunk], F32, tag="h0")
        h1 = hs1_pool.tile([63, w_chunk], F32, tag="h1")
        nc.sync.dma_start(out=h0, in_=hs_d[0:128, i * w_chunk:(i + 1) * w_chunk])
        nc.sync.dma_start(out=h1, in_=hs_d[128:191, i * w_chunk:(i + 1) * w_chunk])
        pacc = pacc_pool.tile([MB, n_pad], F32, tag="pacc")
        for j in range(nj):
            jj = i * nj + j
            p = pt_pool.tile([128, CH], F32, tag="pt")
            nc.tensor.transpose(
                p[:, 0:128], h0[:, j * 128:(j + 1) * 128], ident)
            nc.tensor.transpose(
                p[:, 128:191], h1[:, j * 128:(j + 1) * 128],
                ident[0:63, 0:63])
            ht = hsT_tiles[jj % len(hsT_tiles)]
            if jj % 3 == 2:
                nc.scalar.copy(ht[:, 0:CH], p)
            else:
                nc.vector.tensor_copy(ht[:, 0:CH], p)
            nc.tensor.matmul(
                pacc,
                lhsT=mst_sb[:, jj, :],
                rhs=ht[:, 0:n_pad],
                start=(j == 0),
                stop=(j == nj - 1),
            )
        if i == 0:
            nc.vector.tensor_copy(acc_sb, pacc)
        else:
            nc.vector.tensor_add(acc_sb, acc_sb, pacc)
    nc.sync.dma_start(out=out_d, in_=acc_sb[:, 0:CH])


def build_nc(hw_c=HW_C, w_chunk=4096, reps=1, num_devices=N_CORES, mode="f32r",
             n_ht=4):
    nc = bass.Bass("TRN2", target_bir_lowering=False, debug=False,
                   num_devices=num_devices)
    # fp32r operands must be produced "rounded": declare ms (DMA'd verbatim)
    # and the hsT staging tiles (DVE/ACT copies do the rounding) as float32r.
    op_dt = F32R if mode == "f32r" else F32
    hs_d = nc.dram_tensor("hs", [CH, hw_c], F32, kind="ExternalInput").ap()
    mst_d = nc.dram_tensor("mst", [128, hw_c // 128, MB], op_dt,
                           kind="ExternalInput").ap()
    out_d = nc.dram_tensor("out", [MB, CH], F32, kind="ExternalOutput").ap()
    n_pad = 256 if mode == "f32r" else CH

    with tile.TileContext(nc) as tc:
        with (
            tc.tile_pool(name="singles", bufs=1) as singles,
            tc.tile_pool(name="hs0", bufs=3) as hs0_pool,
            tc.tile_pool(name="hs1", bufs=3) as hs1_pool,
            tc.tile_pool(name="pt", bufs=3, space=bass.MemorySpace.PSUM) as pt_pool,
            tc.tile_pool(name="pacc", bufs=1, space=bass.MemorySpace.PSUM) as pacc_pool,
            tc.tile_pool(name="accp", bufs=1) as acc_pool,
        ):
            ident = singles.tile([128, 128], F32)
            make_identity(nc, ident)
            mst_sb = singles.tile([128, hw_c // 128, MB], op_dt)
            nc.sync.dma_start(out=mst_sb, in_=mst_d)
            # Persistent round-robin hsT staging tiles; tail columns beyond CH
            # are zeroed once and never rewritten (copies touch only [:, :CH]).
            hsT_tiles = [singles.tile([128, n_pad], op_dt, name=f"ht{b}",
                                      tag=f"ht{b}")
                         for b in range(n_ht)]
            for t in hsT_tiles:
                nc.vector.memset(t.bitcast(F32) if mode == "f32r" else t, 0.0)

            pools = (hs0_pool, hs1_pool, hsT_tiles, pt_pool, pacc_pool,
                     acc_pool)
            if reps == 1:
                _emit_body(nc, pools, hs_d, mst_sb, out_d, hw_c, w_chunk,
                           mode, ident)
            else:
                with tc.For_i(0, reps, 1) as _i:
                    _emit_body(nc, pools, hs_d, mst_sb, out_d, hw_c, w_chunk,
                               mode, ident)
    return nc


# ---------------------------------------------------------------------------
# Host wrapper
# ---------------------------------------------------------------------------

_NC_CACHE = {}


def _get_nc(**kwargs):
    key = tuple(sorted(kwargs.items()))
    if key not in _NC_CACHE:
        _NC_CACHE[key] = build_nc(**kwargs)
    return _NC_CACHE[key]


def make_in_maps(hs, ms):
    hs = np.asarray(hs, dtype=np.float32)
    ms = np.asarray(ms, dtype=np.float32)
    hsf = hs.reshape(CH, HW)
    msf = ms.reshape(MB, HW)
    in_maps = []
    for c in range(N_CORES):
        sl = slice(c * HW_C, (c + 1) * HW_C)
        hs_c = np.ascontiguousarray(hsf[:, sl])
        ms_c = msf[:, sl]
        # mst[k, j, m] = ms_c[m, 128*j + k]
        mst_c = np.ascontiguousarray(
            ms_c.reshape(MB, HW_C // 128, 128).transpose(2, 1, 0))
        in_maps.append({"hs": hs_c, "mst": mst_c})
    return in_maps


def kernel(hs, ms):
    in_maps = make_in_maps(hs, ms)
    nc = _get_nc()
    res = run_bass_kernel_spmd(nc, in_maps, list(range(N_CORES)))
    out = np.zeros((MB, CH), np.float64)
    for c in range(N_CORES):
        out += res.results[c]["out"].astype(np.float64)
    return out.astype(np.float32)[:, :, None, None]



# revision 2
# speedup vs baseline: 1.4733x; 1.4733x over previous
"""Trainium2 Bass kernel for nn_Conv_spe_12489764897428.

Math: out[m, c] = sum_hw hs[0, c, h, w] * ms[m, 0, h, w]
  == matmul ms_flat[8, HW] @ hs_flat[191, HW].T with HW = 512*512 = 262144.

Sharding: HW (contraction) axis split across 8 cores; each core computes the
full [8, 191] partial over its 32768-wide HW slice; host sums the partials.

v4 design:
  - Host pre-transposes BOTH operands into the exact SBUF layout the PE
    needs (hw on partitions), in fp16: halves HBM traffic vs f32 and removes
    all on-chip transposes/staging copies. hs is additionally laid out
    CHUNK-CONTIGUOUS: each 32-block chunk [128, 32*191] is one contiguous
    1.5 MB DRAM region, so the DMA engines stream sequential addresses.
  - Device: per 128-hw block j, one fp16 matmul accumulating into a single
    PSUM bank over all 256 blocks (start on first, stop on last):
        pacc[8, 191] += ms_prep[:, j*8:(j+1)*8].T @ hs_chunk[:, ...]
    PE cost ~199 cyc/block (8 ld + 191 mov) - well under the DMA rate.
  - hs chunks alternate between the SP and ACT HWDGE queues.
  - The last chunk is DMA'd in tapering column-sliced pieces so the final
    matmuls (and the end-of-rep drain) finish right after the stream ends.
  - Per-rep result drain (PSUM->SBUF->DRAM) is software-pipelined into the
    START of the next rep, overlapped with its hs stream; an epilogue after
    the loop drains the final rep.
"""

import numpy as np

import concourse.bass as bass
import concourse.mybir as mybir
import concourse.tile as tile
from concourse.bass_utils import run_bass_kernel_spmd
from concourse.vector_clock import ScopedClock

N_CORES = 8
CH = 191                 # hs channels (band_hs)
MB = 8                   # ms bands (band_ms)
HW = 512 * 512
HW_C = HW // N_CORES     # 32768 hw positions per core
NBLK = HW_C // 128       # 256 blocks of 128 hw positions
CB = 32                  # blocks per chunk (fixed in the DRAM layout)
NCHUNK = NBLK // CB      # 8 chunks
TAPER = [16, 8, 4, 2, 2]  # sub-DMA split (blocks) of the last chunk
F32 = mybir.dt.float32
F16 = mybir.dt.float16

# ---------------------------------------------------------------------------
# Workarounds: walrus in this environment encodes at most ONE sync-wait per
# instruction (CTRL and S3_LW struct lowerings reject more with "Too many
# sync wait commands"). Tile freely attaches several. Split them: keep one
# wait on the instruction, hoist the rest onto same-engine NOPs placed just
# before it in the scheduled order.
# ---------------------------------------------------------------------------

_orig_lower_ordered_insts = tile.TileContext._lower_ordered_insts


def _split_multi_waits(nc, blocks):
    for bb, insts in list(blocks.items()):
        new_list = []
        changed = False
        for inst in insts:
            si = getattr(inst, "sync_info", None)
            waits = list(si.on_wait) if si is not None and si.on_wait else []
            if len(waits) > 1:
                si.on_wait = [waits[0]]
                for w in waits[1:]:
                    nop = mybir.InstNoOp(
                        name=nc.get_next_instruction_name(),
                        engine=inst.engine,
                        ins=[],
                        outs=[],
                        sync_info=mybir.SyncInfo(on_wait=[w], on_update=[]),
                        bass_nofuse=True,
                    )
                    nc.register_instruction(nop)
                    new_list.append(nop)
                changed = True
            new_list.append(inst)
        if changed:
            blocks[bb] = new_list


def _patched_lower_ordered_insts(self, postordered_blocks):
    _split_multi_waits(self.nc, postordered_blocks)
    return _orig_lower_ordered_insts(self, postordered_blocks)


tile.TileContext._lower_ordered_insts = _patched_lower_ordered_insts


def _patched_drain_and_barrier(self, tick_clock, wait_clock):
    nop_inst = self.nc.sync.nop(nofuse=True, hint="tail_drain_waits")
    wait_clock.add_sem_waits(
        nop_inst.ins, ScopedClock({None: tick_clock.global_clock})
    )
    si = nop_inst.ins.sync_info
    waits = list(si.on_wait) if si is not None and si.on_wait else []
    if len(waits) > 1:
        si.on_wait = [waits[0]]
        for w in waits[1:]:
            extra = self.nc.sync.nop(nofuse=True, hint="tail_drain_waits")
            esi = extra.ins.sync_info
            if esi is None:
                extra.ins.sync_info = mybir.SyncInfo(on_wait=[w], on_update=[])
            else:
                esi.on_wait = [w]

    self.nc.sync.drain()

    self.nc.all_engine_barrier()
    assert self.sems is not None
    popped = self.nc._tile_sem_poison_stack.pop()
    assert popped is self._sem_poison
    self.nc.clear_and_free_semaphores(list(self.sems.allocated().values()))
    self.nc.all_engine_barrier()


tile.TileContext._drain_and_barrier = _patched_drain_and_barrier


# ---------------------------------------------------------------------------
# Device kernel
# ---------------------------------------------------------------------------


def _emit_body(nc, tiles, hs_pool, ms_pool, hs_d, ms_d, out_d):
    """One rep; the previous rep's result is drained at the start (software
    pipelining), see module docstring."""
    pacc, acc_sb = tiles

    mst_sb = ms_pool.tile([128, NBLK * MB], F16, tag="mst")
    nc.scalar.dma_start(out=mst_sb, in_=ms_d)
    # Previous rep's result: PSUM -> SBUF on DVE, then DRAM via ACT's queue.
    nc.vector.tensor_copy(acc_sb, pacc)
    nc.scalar.dma_start(out=out_d, in_=acc_sb)

    state = {"jj": 0, "qi": 0}

    def emit_chunk(q, sub_blocks):
        """DMA (column-sliced pieces of) chunk q and emit its matmuls."""
        b0 = 0
        for scb in sub_blocks:
            h = hs_pool.tile([128, CB * CH], F16, tag="h")
            eng = nc.sync if state["qi"] % 2 == 0 else nc.scalar
            state["qi"] += 1
            eng.dma_start(
                out=h[:, 0:scb * CH],
                in_=hs_d[q * 128:(q + 1) * 128,
                         b0 * CH:(b0 + scb) * CH])
            for j in range(scb):
                jj = state["jj"]
                nc.tensor.matmul(
                    pacc,
                    lhsT=mst_sb[:, jj * MB:(jj + 1) * MB],
                    rhs=h[:, j * CH:(j + 1) * CH],
                    start=(jj == 0),
                    stop=(jj == NBLK - 1),
                )
                state["jj"] += 1
            b0 += scb

    for q in range(NCHUNK - 1):
        emit_chunk(q, [CB])
    emit_chunk(NCHUNK - 1, TAPER)


def _emit_tail(nc, tiles, out_d):
    pacc, acc_sb = tiles
    nc.vector.tensor_copy(acc_sb, pacc)
    nc.scalar.dma_start(out=out_d, in_=acc_sb)


def build_nc(reps=1, num_devices=N_CORES, hs_bufs=4):
    nc = bass.Bass("TRN2", target_bir_lowering=False, debug=False,
                   num_devices=num_devices)
    hs_d = nc.dram_tensor("hs", [NCHUNK * 128, CB * CH], F16,
                          kind="ExternalInput").ap()
    ms_d = nc.dram_tensor("mst", [128, NBLK * MB], F16,
                          kind="ExternalInput").ap()
    out_d = nc.dram_tensor("out", [MB, CH], F32, kind="ExternalOutput").ap()

    with tile.TileContext(nc) as tc:
        with (
            tc.tile_pool(name="singles", bufs=1) as singles,
            tc.tile_pool(name="hs", bufs=hs_bufs) as hs_pool,
            tc.tile_pool(name="ms", bufs=2) as ms_pool,
            tc.tile_pool(name="pacc", bufs=1, space=bass.MemorySpace.PSUM) as pacc_pool,
        ):
            pacc = pacc_pool.tile([MB, CH], F32, tag="pacc")
            acc_sb = singles.tile([MB, CH], F32, tag="acc")
            # First body iteration reads pacc before any matmul wrote it.
            nc.vector.memset(pacc, 0.0)
            tiles = (pacc, acc_sb)
            if reps == 1:
                _emit_body(nc, tiles, hs_pool, ms_pool, hs_d, ms_d, out_d)
            else:
                with tc.For_i(0, reps, 1) as _i:
                    _emit_body(nc, tiles, hs_pool, ms_pool, hs_d, ms_d, out_d)
            _emit_tail(nc, tiles, out_d)
    return nc


# ---------------------------------------------------------------------------
# Host wrapper
# ---------------------------------------------------------------------------

_NC_CACHE = {}


def _get_nc(**kwargs):
    key = tuple(sorted(kwargs.items()))
    if key not in _NC_CACHE:
        _NC_CACHE[key] = build_nc(**kwargs)
    return _NC_CACHE[key]


def make_in_maps(hs, ms):
    hs = np.asarray(hs, dtype=np.float32)
    ms = np.asarray(ms, dtype=np.float32)
    # hs_prep[c][q*128+k, j*CH+t] = hs[t, c*HW_C + (q*CB + j)*128 + k]
    hs_prep = np.ascontiguousarray(
        hs.reshape(CH, N_CORES, NCHUNK, CB, 128).transpose(1, 2, 4, 3, 0)
    ).astype(np.float16).reshape(N_CORES, NCHUNK * 128, CB * CH)
    # ms_prep[c][k, j*MB + m] = ms[m, c*HW_C + j*128 + k]
    ms_prep = np.ascontiguousarray(
        ms.reshape(MB, N_CORES, NBLK, 128).transpose(1, 3, 2, 0)
    ).astype(np.float16).reshape(N_CORES, 128, NBLK * MB)
    return [{"hs": hs_prep[c], "mst": ms_prep[c]} for c in range(N_CORES)]


def kernel(hs, ms):
    in_maps = make_in_maps(hs, ms)
    nc = _get_nc()
    res = run_bass_kernel_spmd(nc, in_maps, list(range(N_CORES)))
    out = np.zeros((MB, CH), np.float64)
    for c in range(N_CORES):
        out += res.results[c]["out"].astype(np.float64)
    return out.astype(np.float32)[:, :, None, None]
